# revision 10
# baseline (speedup 1.0000x reference)
"""BGRL forward pass on Trainium2 (8 NeuronCores), Bass/Tile.

Math used (A = edge-weighted adjacency, aggregation commutes with the
dense matmul since both are linear):

    gcn(x, W, b) = A @ (x W) + b = (A @ x) @ W + b

so we aggregate the concatenated feature table XP = [x | perb] ONCE
(one gather pass over the edges) and apply the small [D,D] weights to
the aggregates afterwards.  That halves the dominant random-gather
traffic versus aggregating the four h = x@W products.

Sharding: nodes (destination rows) are sharded contiguously across the
8 cores; each core's edges (binned by destination row) are packed into
"groups" of <= BPG*128 edge slots covering <= 128 consecutive rows.
Within a group, segment-sum is a one-hot matmul accumulated in PSUM:

    selw[e, r] = (rowrel[e] == r) * w[e]      (one DVE op per block)
    agg[r, f] += selw.T @ gathered[e, f]      (PE, PSUM accumulate)

The only cross-core exchange is a [128, 4] AllReduce for the BatchNorm
statistics of the predictor; per-node loss terms are partial-summed on
each core and combined on the host during unshard.
"""

import os
import numpy as np
from contextlib import ExitStack

import concourse.bass as bass
import concourse.tile as tile
from concourse import bacc, mybir
from concourse.masks import make_identity

P = 128          # partitions == feature dim D
D = 128
BPG = 16         # blocks (of 128 edge slots) per group
EPS_BN = 1e-5
EPS_NORM = 1e-12

f32 = mybir.dt.float32
i32 = mybir.dt.int32
AF = mybir.ActivationFunctionType
OP = mybir.AluOpType


# ----------------------------------------------------------------------------
# Host-side preprocessing: bin edges by destination row into per-core groups.
# ----------------------------------------------------------------------------

def _prepare(edge_row, edge_col, edge_w, n, n_cores):
    rows_per_core = n // n_cores
    assert rows_per_core * n_cores == n
    epg = P * BPG

    order = np.argsort(edge_row, kind="stable")
    r_sorted = edge_row[order]
    c_sorted = edge_col[order]
    w_sorted = edge_w[order]
    counts = np.bincount(edge_row, minlength=n)
    row_start = np.concatenate([[0], np.cumsum(counts)])
    assert counts.max() <= epg, "single row exceeds group capacity"

    core_groups = []
    for k in range(n_cores):
        r0, r1 = k * rows_per_core, (k + 1) * rows_per_core
        groups = []
        r = r0
        while r < r1:
            re, e = r, 0
            while re < r1 and (re - r) < P and e + counts[re] <= epg:
                e += counts[re]
                re += 1
            groups.append((r, re))
            r = re
        core_groups.append(groups)
    ng = max(len(g) for g in core_groups)

    per_core = []
    trash = rows_per_core  # scatter target for padded rows
    for k in range(n_cores):
        r0 = k * rows_per_core
        groups = core_groups[k]
        cols = np.zeros((ng, P, BPG), np.int32)
        wts = np.zeros((ng, P, BPG), np.float32)
        rowrel = np.full((ng, P, BPG), -1.0, np.float32)
        drows = np.full((ng, P), trash, np.int32)
        grows = np.zeros((ng, P), np.int32)
        maskT = np.zeros((P, ng), np.float32)
        for g, (rs, re) in enumerate(groups):
            e0, e1 = row_start[rs], row_start[re]
            m = e1 - e0
            ii = np.arange(m)
            p, j = ii % P, ii // P
            cols[g, p, j] = c_sorted[e0:e1]
            wts[g, p, j] = w_sorted[e0:e1]
            rowrel[g, p, j] = (r_sorted[e0:e1] - rs).astype(np.float32)
            nrows = re - rs
            pr = np.arange(nrows)
            drows[g, :nrows] = (rs - r0) + pr
            grows[g, :nrows] = rs + pr
            maskT[:nrows, g] = 1.0
        npad = float(P * ng - rows_per_core)
        per_core.append(dict(
            cols=np.ascontiguousarray(cols.transpose(1, 0, 2).reshape(P, ng * BPG)),
            wts=np.ascontiguousarray(wts.transpose(1, 0, 2).reshape(P, ng * BPG)),
            rowrel=np.ascontiguousarray(
                rowrel.transpose(1, 0, 2).reshape(P, ng * BPG)),
            drows=np.ascontiguousarray(drows.T),
            grows=np.ascontiguousarray(grows.T),
            maskT=np.ascontiguousarray(maskT),
            npad=np.full((P, 1), npad, np.float32),
        ))
    return ng, rows_per_core, per_core


# ----------------------------------------------------------------------------
# Device program (identical on every core; per-core behaviour comes from the
# per-core input tensors).
# ----------------------------------------------------------------------------

def _build(ng, n, rows_per_core, n_cores, alpha):
    nc = bacc.Bacc(None, target_bir_lowering=False, debug=False,
                   num_devices=n_cores)

    xp_d = nc.dram_tensor("xp", [n, 2 * D], f32, kind="ExternalInput")
    cols_d = nc.dram_tensor("cols", [P, ng * BPG], i32, kind="ExternalInput")
    wts_d = nc.dram_tensor("wts", [P, ng * BPG], f32, kind="ExternalInput")
    rowrel_d = nc.dram_tensor("rowrel", [P, ng * BPG], f32, kind="ExternalInput")
    drows_d = nc.dram_tensor("drows", [P, ng], i32, kind="ExternalInput")
    grows_d = nc.dram_tensor("grows", [P, ng], i32, kind="ExternalInput")
    maskT_d = nc.dram_tensor("maskT", [P, ng], f32, kind="ExternalInput")
    npad_d = nc.dram_tensor("npad", [P, 1], f32, kind="ExternalInput")
    iota_d = nc.dram_tensor("iota", [P, P], f32, kind="ExternalInput")
    w_on_d = nc.dram_tensor("w_on", [D, D], f32, kind="ExternalInput")
    w_tg_d = nc.dram_tensor("w_tg", [D, D], f32, kind="ExternalInput")
    w1_d = nc.dram_tensor("w1", [D, D], f32, kind="ExternalInput")
    w2_d = nc.dram_tensor("w2", [D, D], f32, kind="ExternalInput")
    b_on_d = nc.dram_tensor("b_on", [D], f32, kind="ExternalInput")
    b_tg_d = nc.dram_tensor("b_tg", [D], f32, kind="ExternalInput")
    b1_d = nc.dram_tensor("b1", [D], f32, kind="ExternalInput")
    b2_d = nc.dram_tensor("b2", [D], f32, kind="ExternalInput")
    bng_d = nc.dram_tensor("bng", [D], f32, kind="ExternalInput")
    bnb_d = nc.dram_tensor("bnb", [D], f32, kind="ExternalInput")

    embed_d = nc.dram_tensor("embed", [rows_per_core + 1, D], f32,
                             kind="ExternalOutput")
    loss_d = nc.dram_tensor("loss", [1, 1], f32, kind="ExternalOutput")
    debug = bool(int(os.environ.get("BGRL_DEBUG", "0")))
    if debug:
        xprd_d = nc.dram_tensor("xprd", [P, ng * 2 * D], f32,
                                kind="ExternalOutput")
        embt_d = nc.dram_tensor("embt", [P, ng * D], f32,
                                kind="ExternalOutput")

    with tile.TileContext(nc) as tc, ExitStack() as ctx:
        const = ctx.enter_context(tc.tile_pool(name="const", bufs=1))
        stash = ctx.enter_context(tc.tile_pool(name="stash", bufs=1))
        dramp = ctx.enter_context(tc.tile_pool(name="dram", bufs=1, space="DRAM"))

        def load_const(shape, dtype, src_ap, tag):
            t = const.tile(shape, dtype, tag=tag)
            nc.sync.dma_start(out=t[:], in_=src_ap)
            return t

        iota_t = load_const([P, P], f32, iota_d[:], "iota")
        won_t = load_const([D, D], f32, w_on_d[:], "won")
        wtg_t = load_const([D, D], f32, w_tg_d[:], "wtg")
        w1_t = load_const([D, D], f32, w1_d[:], "w1")
        w2_t = load_const([D, D], f32, w2_d[:], "w2")
        bon_t = load_const([D, 1], f32, b_on_d[:, None], "bon")
        btg_t = load_const([D, 1], f32, b_tg_d[:, None], "btg")
        b1_t = load_const([D, 1], f32, b1_d[:, None], "b1")
        b2_t = load_const([D, 1], f32, b2_d[:, None], "b2")
        bng_t = load_const([D, 1], f32, bng_d[:, None], "bng")
        bnb_t = load_const([D, 1], f32, bnb_d[:, None], "bnb")
        cols_t = load_const([P, ng * BPG], i32, cols_d[:], "cols")
        wts_t = load_const([P, ng * BPG], f32, wts_d[:], "wtsm")
        rowrel_t = load_const([P, ng * BPG], f32, rowrel_d[:], "rrm")
        drows_t = load_const([P, ng], i32, drows_d[:], "drows")
        grows_t = load_const([P, ng], i32, grows_d[:], "grows")
        maskT_t = load_const([P, ng], f32, maskT_d[:], "maskT")
        npad_t = load_const([P, 1], f32, npad_d[:], "npad")

        ident_t = const.tile([P, P], f32, tag="ident")
        make_identity(nc, ident_t[:])
        ones_t = const.tile([P, 1], f32, tag="ones")
        nc.vector.memset(ones_t[:], 1.0)
        epsbn_t = const.tile([P, 1], f32, tag="epsbn")
        nc.vector.memset(epsbn_t[:], float(EPS_BN))

        h1x_st = stash.tile([P, ng * P], f32, tag="h1x")
        h1y_st = stash.tile([P, ng * P], f32, tag="h1y")
        tgx_st = stash.tile([P, ng * P], f32, tag="tgx")
        tgy_st = stash.tile([P, ng * P], f32, tag="tgy")
        bnsums_st = stash.tile([P, 4 * ng], f32, tag="bnsums")
        lred_st = stash.tile([P, 6 * ng], f32, tag="lred")

        # ------------------------------------------------------- phase A ----
        with tc.tile_pool(name="gath", bufs=2 * BPG) as gpool, \
             tc.tile_pool(name="xpr", bufs=2) as xpool, \
             tc.tile_pool(name="selw", bufs=3) as spool, \
             tc.tile_pool(name="wa", bufs=3) as wa, \
             tc.tile_pool(name="agg_ps", bufs=2, space="PSUM") as agg_pp, \
             tc.tile_pool(name="aggT_ps", bufs=1, space="PSUM") as aggT_pp, \
             tc.tile_pool(name="prod_ps", bufs=2, space="PSUM") as prod_pp, \
             tc.tile_pool(name="h1_ps", bufs=2, space="PSUM") as h1_pp, \
             tc.tile_pool(name="eT_ps", bufs=1, space="PSUM") as eT_pp:
            for g in range(ng):
                # HW indirect DMA supports exactly one index per partition:
                # one gather per 128-edge block.
                gath = []
                for j in range(BPG):
                    cj = g * BPG + j
                    gt = gpool.tile([P, 2 * D], f32, tag="gath")
                    nc.gpsimd.indirect_dma_start(
                        out=gt[:], out_offset=None, in_=xp_d[:],
                        in_offset=bass.IndirectOffsetOnAxis(
                            ap=cols_t[:, cj:cj + 1], axis=0))
                    gath.append(gt)
                xpr = xpool.tile([P, 2 * D], f32, tag="xpr")
                nc.gpsimd.indirect_dma_start(
                    out=xpr[:], out_offset=None, in_=xp_d[:],
                    in_offset=bass.IndirectOffsetOnAxis(
                        ap=grows_t[:, g:g + 1], axis=0))

                agg_ps = agg_pp.tile([P, 2 * D], f32, tag="agg")
                for j in range(BPG):
                    cj = g * BPG + j
                    selw = spool.tile([P, P], f32, tag="selw")
                    nc.vector.tensor_scalar(
                        out=selw[:], in0=iota_t[:],
                        scalar1=rowrel_t[:, cj:cj + 1],
                        scalar2=wts_t[:, cj:cj + 1],
                        op0=OP.is_equal, op1=OP.mult)
                    nc.tensor.matmul(
                        out=agg_ps[:], lhsT=selw[:],
                        rhs=gath[j][:],
                        start=(j == 0), stop=(j == BPG - 1))

                agg_s = wa.tile([P, 2 * D], f32, tag="agg_s")
                nc.scalar.activation(out=agg_s[:], in_=agg_ps[:], func=AF.Copy)
                aggx2_s = wa.tile([P, D], f32, tag="aggx2")
                nc.vector.tensor_tensor(out=aggx2_s[:], in0=agg_s[:, :D],
                                        in1=agg_s[:, D:], op=OP.add)

                aggT_ps = aggT_pp.tile([P, 2 * D], f32, tag="aggT")
                nc.tensor.transpose(out=aggT_ps[:, :D], in_=agg_s[:, :D],
                                    identity=ident_t[:])
                nc.tensor.transpose(out=aggT_ps[:, D:], in_=aggx2_s[:],
                                    identity=ident_t[:])
                aggxT_s = wa.tile([P, D], f32, tag="aggxT")
                nc.scalar.activation(out=aggxT_s[:], in_=aggT_ps[:, :D],
                                     func=AF.Copy)
                aggx2T_s = wa.tile([P, D], f32, tag="aggx2T")
                nc.scalar.activation(out=aggx2T_s[:], in_=aggT_ps[:, D:],
                                     func=AF.Copy)

                # [onx | tgy] from agg(x);  [ony | tgx] from agg(x2)
                prodA_ps = prod_pp.tile([P, 2 * D], f32, tag="prod")
                nc.tensor.matmul(out=prodA_ps[:, :D], lhsT=won_t[:],
                                 rhs=aggxT_s[:], start=True, stop=True)
                nc.tensor.matmul(out=prodA_ps[:, D:], lhsT=wtg_t[:],
                                 rhs=aggxT_s[:], start=True, stop=True)
                prodB_ps = prod_pp.tile([P, 2 * D], f32, tag="prod")
                nc.tensor.matmul(out=prodB_ps[:, :D], lhsT=won_t[:],
                                 rhs=aggx2T_s[:], start=True, stop=True)
                nc.tensor.matmul(out=prodB_ps[:, D:], lhsT=wtg_t[:],
                                 rhs=aggx2T_s[:], start=True, stop=True)

                gs = slice(g * P, (g + 1) * P)
                onx_s = wa.tile([P, D], f32, tag="onx")
                nc.scalar.activation(out=onx_s[:], in_=prodA_ps[:, :D],
                                     func=AF.Identity, bias=bon_t[:, :1])
                nc.scalar.activation(out=tgy_st[:, gs], in_=prodA_ps[:, D:],
                                     func=AF.Identity, bias=btg_t[:, :1])
                ony_s = wa.tile([P, D], f32, tag="ony")
                nc.scalar.activation(out=ony_s[:], in_=prodB_ps[:, :D],
                                     func=AF.Identity, bias=bon_t[:, :1])
                nc.scalar.activation(out=tgx_st[:, gs], in_=prodB_ps[:, D:],
                                     func=AF.Identity, bias=btg_t[:, :1])

                h1_ps = h1_pp.tile([P, 2 * D], f32, tag="h1")
                nc.tensor.matmul(out=h1_ps[:, :D], lhsT=w1_t[:], rhs=onx_s[:],
                                 start=True, stop=True)
                nc.tensor.matmul(out=h1_ps[:, D:], lhsT=w1_t[:], rhs=ony_s[:],
                                 start=True, stop=True)
                nc.scalar.activation(out=h1x_st[:, gs], in_=h1_ps[:, :D],
                                     func=AF.Identity, bias=b1_t[:, :1],
                                     accum_out=bnsums_st[:, 4 * g:4 * g + 1])
                nc.scalar.activation(out=h1y_st[:, gs], in_=h1_ps[:, D:],
                                     func=AF.Identity, bias=b1_t[:, :1],
                                     accum_out=bnsums_st[:, 4 * g + 2:4 * g + 3])
                junk = wa.tile([P, D], f32, tag="junk")
                nc.scalar.activation(out=junk[:], in_=h1x_st[:, gs],
                                     func=AF.Square,
                                     accum_out=bnsums_st[:, 4 * g + 1:4 * g + 2])
                junk2 = wa.tile([P, D], f32, tag="junk2")
                nc.scalar.activation(out=junk2[:], in_=h1y_st[:, gs],
                                     func=AF.Square,
                                     accum_out=bnsums_st[:, 4 * g + 3:4 * g + 4])

                # embed = x2 + online_y  (node-major), scattered to local rows
                x2r = wa.tile([P, D], f32, tag="x2r")
                nc.vector.tensor_tensor(out=x2r[:], in0=xpr[:, :D],
                                        in1=xpr[:, D:], op=OP.add)
                eT_ps = eT_pp.tile([P, D], f32, tag="eT")
                nc.tensor.transpose(out=eT_ps[:], in_=ony_s[:],
                                    identity=ident_t[:])
                embed_s = wa.tile([P, D], f32, tag="embed_s")
                nc.vector.tensor_tensor(out=embed_s[:], in0=x2r[:],
                                        in1=eT_ps[:], op=OP.add)
                nc.gpsimd.indirect_dma_start(
                    out=embed_d[:],
                    out_offset=bass.IndirectOffsetOnAxis(
                        ap=drows_t[:, g:g + 1], axis=0),
                    in_=embed_s[:], in_offset=None)
                if debug:
                    nc.sync.dma_start(
                        out=xprd_d[:, g * 2 * D:(g + 1) * 2 * D], in_=xpr[:])
                    x2T_ps = eT_pp.tile([P, D], f32, tag="eT")
                    nc.tensor.transpose(out=x2T_ps[:], in_=x2r[:],
                                        identity=ident_t[:])
                    embT_s = wa.tile([P, D], f32, tag="embT")
                    nc.vector.tensor_tensor(out=embT_s[:], in0=ony_s[:],
                                            in1=x2T_ps[:], op=OP.add)
                    nc.sync.dma_start(
                        out=embt_d[:, g * D:(g + 1) * D], in_=embT_s[:])

        # ------------------------------------- BN stats + AllReduce ---------
        stats = ctx.enter_context(tc.tile_pool(name="stats", bufs=1))
        with tc.tile_pool(name="c_ps", bufs=1, space="PSUM") as c_pp:
            c_ps = c_pp.tile([P, 1], f32, tag="c")
            nc.tensor.matmul(out=c_ps[:], lhsT=w1_t[:], rhs=bon_t[:, :1],
                             start=True, stop=True)
            c_s = stats.tile([P, 1], f32, tag="c_s")
            nc.scalar.activation(out=c_s[:], in_=c_ps[:], func=AF.Identity,
                                 bias=b1_t[:, :1])
        c2_s = stats.tile([P, 1], f32, tag="c2_s")
        nc.vector.tensor_tensor(out=c2_s[:], in0=c_s[:], in1=c_s[:], op=OP.mult)

        bn4_t = stats.tile([P, 4], f32, tag="bn4")
        bnv = bnsums_st[:].rearrange("p (g k) -> p k g", k=4)
        for k in range(4):
            nc.vector.reduce_sum(out=bn4_t[:, k:k + 1], in_=bnv[:, k:k + 1, :],
                                 axis=mybir.AxisListType.X)
        npc = stats.tile([P, 1], f32, tag="npc")
        nc.vector.tensor_tensor(out=npc[:], in0=c_s[:], in1=npad_t[:], op=OP.mult)
        npc2 = stats.tile([P, 1], f32, tag="npc2")
        nc.vector.tensor_tensor(out=npc2[:], in0=c2_s[:], in1=npad_t[:], op=OP.mult)
        for k, corr in ((0, npc), (1, npc2), (2, npc), (3, npc2)):
            nc.vector.tensor_tensor(out=bn4_t[:, k:k + 1], in0=bn4_t[:, k:k + 1],
                                    in1=corr[:], op=OP.subtract)

        cc_in = dramp.tile([P, 4], f32, tag="cc_in")
        cc_out = dramp.tile([P, 4], f32, tag="cc_out",
                            addr_space="Shared" if n_cores > 4 else "Local")
        nc.gpsimd.dma_start(out=cc_in[:], in_=bn4_t[:])
        if int(os.environ.get("BGRL_NOCC", "0")):
            nc.gpsimd.dma_start(out=cc_out[:], in_=cc_in[:])
        else:
            nc.gpsimd.collective_compute(
                "AllReduce", OP.add,
                replica_groups=[list(range(n_cores))],
                ins=[cc_in[:].opt()], outs=[cc_out[:].opt()])
        bnred_t = stats.tile([P, 4], f32, tag="bnred")
        nc.gpsimd.dma_start(out=bnred_t[:], in_=cc_out[:])

        # scale/shift per side; also alpha-scaled copies for PReLU max trick
        sc, sh, sca, sha = {}, {}, {}, {}
        for side, (ks, kq) in (("x", (0, 1)), ("y", (2, 3))):
            mu = stats.tile([P, 1], f32, tag=f"mu{side}")
            nc.vector.tensor_scalar(out=mu[:], in0=bnred_t[:, ks:ks + 1],
                                    scalar1=1.0 / n, scalar2=None, op0=OP.mult)
            ex2 = stats.tile([P, 1], f32, tag=f"ex2{side}")
            nc.vector.tensor_scalar(out=ex2[:], in0=bnred_t[:, kq:kq + 1],
                                    scalar1=1.0 / n, scalar2=None, op0=OP.mult)
            mu2 = stats.tile([P, 1], f32, tag=f"mu2{side}")
            nc.vector.tensor_tensor(out=mu2[:], in0=mu[:], in1=mu[:], op=OP.mult)
            var = stats.tile([P, 1], f32, tag=f"var{side}")
            nc.vector.tensor_tensor(out=var[:], in0=ex2[:], in1=mu2[:],
                                    op=OP.subtract)
            sd = stats.tile([P, 1], f32, tag=f"sd{side}")
            nc.scalar.activation(out=sd[:], in_=var[:], func=AF.Sqrt,
                                 bias=epsbn_t[:, :1])
            inv = stats.tile([P, 1], f32, tag=f"inv{side}")
            nc.vector.reciprocal(out=inv[:], in_=sd[:])
            s_t = stats.tile([P, 1], f32, tag=f"sc{side}")
            nc.vector.tensor_tensor(out=s_t[:], in0=bng_t[:], in1=inv[:],
                                    op=OP.mult)
            musc = stats.tile([P, 1], f32, tag=f"musc{side}")
            nc.vector.tensor_tensor(out=musc[:], in0=mu[:], in1=s_t[:], op=OP.mult)
            t_t = stats.tile([P, 1], f32, tag=f"sh{side}")
            nc.vector.tensor_tensor(out=t_t[:], in0=bnb_t[:], in1=musc[:],
                                    op=OP.subtract)
            sa_t = stats.tile([P, 1], f32, tag=f"sca{side}")
            nc.vector.tensor_scalar(out=sa_t[:], in0=s_t[:], scalar1=alpha,
                                    scalar2=None, op0=OP.mult)
            ta_t = stats.tile([P, 1], f32, tag=f"sha{side}")
            nc.vector.tensor_scalar(out=ta_t[:], in0=t_t[:], scalar1=alpha,
                                    scalar2=None, op0=OP.mult)
            sc[side], sh[side], sca[side], sha[side] = s_t, t_t, sa_t, ta_t

        # ------------------------------------------------------- phase B ----
        with tc.tile_pool(name="wb", bufs=3) as wb, \
             tc.tile_pool(name="px_ps", bufs=2, space="PSUM") as px_pp, \
             tc.tile_pool(name="red_ps", bufs=2, space="PSUM") as red_pp:
            for g in range(ng):
                gs = slice(g * P, (g + 1) * P)
                hs = {}
                for side, st in (("x", h1x_st), ("y", h1y_st)):
                    z1 = wb.tile([P, D], f32, tag="z1")
                    nc.scalar.activation(out=z1[:], in_=st[:, gs],
                                         func=AF.Identity,
                                         scale=sc[side][:, :1],
                                         bias=sh[side][:, :1])
                    z2 = wb.tile([P, D], f32, tag="z2")
                    nc.scalar.activation(out=z2[:], in_=st[:, gs],
                                         func=AF.Identity,
                                         scale=sca[side][:, :1],
                                         bias=sha[side][:, :1])
                    h_t = wb.tile([P, D], f32, tag=f"h{side}")
                    nc.vector.tensor_tensor(out=h_t[:], in0=z1[:], in1=z2[:],
                                            op=OP.max)
                    hs[side] = h_t

                px_ps = px_pp.tile([P, 2 * D], f32, tag="px")
                nc.tensor.matmul(out=px_ps[:, :D], lhsT=w2_t[:], rhs=hs["x"][:],
                                 start=True, stop=True)
                nc.tensor.matmul(out=px_ps[:, D:], lhsT=w2_t[:], rhs=hs["y"][:],
                                 start=True, stop=True)
                px_s = wb.tile([P, D], f32, tag="px_s")
                nc.scalar.activation(out=px_s[:], in_=px_ps[:, :D],
                                     func=AF.Identity, bias=b2_t[:, :1])
                py_s = wb.tile([P, D], f32, tag="py_s")
                nc.scalar.activation(out=py_s[:], in_=px_ps[:, D:],
                                     func=AF.Identity, bias=b2_t[:, :1])

                prod = wb.tile([P, 6 * D], f32, tag="prodsb")
                for kk, (a_t, b_ap) in enumerate((
                        (px_s[:], px_s[:]),
                        (px_s[:], tgx_st[:, gs]),
                        (tgx_st[:, gs], tgx_st[:, gs]),
                        (py_s[:], py_s[:]),
                        (py_s[:], tgy_st[:, gs]),
                        (tgy_st[:, gs], tgy_st[:, gs]))):
                    nc.vector.tensor_tensor(out=prod[:, kk * D:(kk + 1) * D],
                                            in0=a_t, in1=b_ap, op=OP.mult)
                red_ps = red_pp.tile([P, 8], f32, tag="red")
                for kk in range(6):
                    nc.tensor.matmul(out=red_ps[:, kk:kk + 1],
                                     lhsT=prod[:, kk * D:(kk + 1) * D],
                                     rhs=ones_t[:, :1], start=True, stop=True)
                nc.scalar.activation(out=lred_st[:, 6 * g:6 * (g + 1)],
                                     in_=red_ps[:, 0:6], func=AF.Copy)

        # ------------------------------------------------------- phase C ----
        fin = ctx.enter_context(tc.tile_pool(name="fin", bufs=1))
        lv = lred_st[:].rearrange("p (g k) -> p k g", k=6)

        def view(k):
            return lv[:, k:k + 1, :]

        ltot = fin.tile([P, ng], f32, tag="ltot")
        first = True
        for kxx, kxt, ktt in ((0, 1, 2), (3, 4, 5)):
            nx = fin.tile([P, ng], f32, tag="nx")
            nc.scalar.activation(out=nx[:], in_=view(kxx), func=AF.Sqrt)
            nt = fin.tile([P, ng], f32, tag="nt")
            nc.scalar.activation(out=nt[:], in_=view(ktt), func=AF.Sqrt)
            nc.vector.tensor_scalar(out=nx[:], in0=nx[:], scalar1=EPS_NORM,
                                    scalar2=None, op0=OP.max)
            nc.vector.tensor_scalar(out=nt[:], in0=nt[:], scalar1=EPS_NORM,
                                    scalar2=None, op0=OP.max)
            den = fin.tile([P, ng], f32, tag="den")
            nc.vector.tensor_tensor(out=den[:], in0=nx[:], in1=nt[:], op=OP.mult)
            inv = fin.tile([P, ng], f32, tag="invd")
            nc.vector.reciprocal(out=inv[:], in_=den[:])
            cosv = fin.tile([P, ng], f32, tag="cosv")
            nc.vector.tensor_tensor(out=cosv[:], in0=inv[:], in1=view(kxt),
                                    op=OP.mult)
            # 2 - 2*cos
            lpart = fin.tile([P, ng], f32, tag="lpart")
            nc.vector.tensor_scalar(out=lpart[:], in0=cosv[:], scalar1=-2.0,
                                    scalar2=2.0, op0=OP.mult, op1=OP.add)
            if first:
                nc.vector.tensor_copy(out=ltot[:], in_=lpart[:])
                first = False
            else:
                nc.vector.tensor_tensor(out=ltot[:], in0=ltot[:], in1=lpart[:],
                                        op=OP.add)
        nc.vector.tensor_tensor(out=ltot[:], in0=ltot[:], in1=maskT_t[:],
                                op=OP.mult)
        lcol = fin.tile([P, 1], f32, tag="lcol")
        nc.vector.reduce_sum(out=lcol[:], in_=ltot[:], axis=mybir.AxisListType.X)
        with tc.tile_pool(name="l_ps", bufs=1, space="PSUM") as l_pp:
            l_ps = l_pp.tile([1, 1], f32, tag="lps")
            nc.tensor.matmul(out=l_ps[:], lhsT=lcol[:, :1], rhs=ones_t[:, :1],
                             start=True, stop=True)
            l_s = fin.tile([1, 1], f32, tag="ls")
            nc.scalar.activation(out=l_s[:], in_=l_ps[:], func=AF.Copy)
            nc.sync.dma_start(out=loss_d[:], in_=l_s[:])

    return nc


# ----------------------------------------------------------------------------
# Entry point
# ----------------------------------------------------------------------------

_CACHE = {}
_LAST = {}


def _kernel_impl(inputs, n_cores=8, use_sim=False, trace=False):
    x = np.asarray(inputs["x"], np.float32)
    perb = np.asarray(inputs["perb"], np.float32)
    edge_row = np.asarray(inputs["edge_row"], np.int32)
    edge_col = np.asarray(inputs["edge_col"], np.int32)
    edge_w = np.asarray(inputs["edge_w"], np.float32)
    n = x.shape[0]
    assert x.shape[1] == D

    ng, rows_per_core, per_core = _prepare(edge_row, edge_col, edge_w,
                                           n, n_cores)
    alpha = float(np.asarray(inputs["prelu_a"]).reshape(-1)[0])

    key = (n, n_cores, ng, alpha,
           hash(edge_row.tobytes()) ^ hash(edge_col.tobytes()))
    if key in _CACHE:
        nc = _CACHE[key]
    else:
        nc = _build(ng, n, rows_per_core, n_cores, alpha)
        nc.compile()
        _CACHE[key] = nc

    xp = np.ascontiguousarray(np.concatenate([x, perb], axis=1))
    iota = np.ascontiguousarray(
        np.broadcast_to(np.arange(P, dtype=np.float32), (P, P)))
    shared = dict(
        xp=xp, iota=iota,
        w_on=np.asarray(inputs["W_on"], np.float32),
        w_tg=np.asarray(inputs["W_tg"], np.float32),
        w1=np.asarray(inputs["W1"], np.float32),
        w2=np.asarray(inputs["W2"], np.float32),
        b_on=np.asarray(inputs["b_on"], np.float32),
        b_tg=np.asarray(inputs["b_tg"], np.float32),
        b1=np.asarray(inputs["b1"], np.float32),
        b2=np.asarray(inputs["b2"], np.float32),
        bng=np.asarray(inputs["bn_g"], np.float32),
        bnb=np.asarray(inputs["bn_b"], np.float32),
    )
    in_maps = []
    for k in range(n_cores):
        m = dict(shared)
        m.update(per_core[k])
        in_maps.append(m)

    _LAST["nc"] = nc
    _LAST["in_maps"] = in_maps
    _LAST["n_cores"] = n_cores
    if use_sim:
        from concourse.bass_interp import MultiCoreSim
        sim = MultiCoreSim(nc, num_cores=n_cores)
        for c in range(n_cores):
            for name, arr in in_maps[c].items():
                sim.cores[c].tensor(name)[:] = arr
        sim.simulate(check_with_hw=False)
        results = [{"embed": np.array(sim.cores[c].tensor("embed")),
                    "loss": np.array(sim.cores[c].tensor("loss"))}
                   for c in range(n_cores)]
        res_obj = None
    else:
        from concourse.bass_utils import run_bass_kernel_spmd
        if trace:
            try:
                import antenv.axon_hooks  # noqa: F401
            except ImportError:
                trace = False
        res_obj = run_bass_kernel_spmd(nc, in_maps, core_ids=list(range(n_cores)),
                                       trace=trace)
        results = res_obj.results

    embed = np.concatenate(
        [results[k]["embed"][:rows_per_core] for k in range(n_cores)], axis=0)
    loss = np.float32(sum(float(results[k]["loss"][0, 0])
                          for k in range(n_cores)) / n)
    return (embed, loss), res_obj


def kernel(**inputs):
    (embed, loss), _ = _kernel_impl(inputs, n_cores=8, use_sim=False)
    return embed, loss
